# revision 21
# baseline (speedup 1.0000x reference)
"""Trainium2 Bass kernel for nn_CSBrainLLMVQ (CSBrain conv front-end + LLM VQ codebook).

Sharding: data-parallel over batch (4 batches/core x 8 cores). Per core:
  conv chain / GroupNorm / GELU(erf) / rFFT / depthwise pos-conv in fp32
  (feature-on-partition, token-on-free layout; convs as fp32 matmuls).
  The VQ reduction CB2T[dm,c] = sum_llm inp_w[llm,dm]*cb[c,llm] and the
  output table W2f[c,:] = cb[c] @ outp_w.T + outp_b are precomputed on the
  host (pure weight transforms), so the device only runs the front-end and
  the [tok,200]x[200,4096] score contraction. Scores use hi/lo fp16
  compensation (3 terms x 2 partition groups = 6 matmuls per 512-col chunk),
  keeping the fp32-grade argmin exact. The 133-tap depthwise positional conv
  splits by channel group: ch 0..127 run on the TensorEngine as per-tap
  diagonal f16 matmuls (K=M=128, fast-weight-load) accumulating in fp32 PSUM
  across three compensated passes (hi, scaled-lo, denormal w-residual,
  ~1e-6 exact); ch 128..199 run concurrently as an exact fp32 DVE MAC chain
  over a flat 36-wide padded layout. Argmin via one fp32 max8/find_index8
  pass per 128-token tile; output rows are fetched with indirect-DMA gathers
  from the host-precomputed W2f table and written out in f16 (upcast on host).
"""
import numpy as np

B, CH, NP_, PS = 32, 19, 30, 200
DM, LLM, KC = 200, 4096, 4096
EPS = 1e-5
T1 = CH * NP_          # 570 tokens per batch
NB = 4                 # batches per core
TOK = NB * T1          # 2280 tokens per core
NCORES = 8
SQ2I = 0.7071067811865476
PBLK = 724             # pos-conv per-batch block: 4 gutter + 19*36 + 36 zero row

_COMPILED = None


def _tok_tiles():
    out, t0 = [], 0
    while t0 < TOK:
        out.append((t0, min(128, TOK - t0)))
        t0 += 128
    return out


def _n_slices(width=512):
    out, n0 = [], 0
    while n0 < TOK:
        out.append((n0, min(width, TOK - n0)))
        n0 += width
    return out


def _pos_taps():
    """(dy, dx, src_base, dst_base, length) for each of the 133 taps, with a
    full-coverage dy=9 tap first for each engine (overwrite, no memset)."""
    taps = []
    order = [(9, dx) for dx in range(7)] + \
        [(dy, dx) for dy in range(19) if dy != 9 for dx in range(7)]
    for dy, dx in order:
        d = dy - 9
        ho, hn, hs = max(0, -d), 19 - abs(d), max(0, d)
        taps.append((dy, dx, 4 + hs * 36 + dx - 3, 4 + ho * 36, hn * 36))
    return taps


def _split_taps():
    """Greedy split balancing projected engine-busy time. DVE runs taps as
    native STT MACs; the GpSimd path runs them as ACT-premultiplied
    tensor-tensor adds (Pool ucode only supports TensorTensor)."""
    taps = _pos_taps()
    DVE_NS = 1.042                  # ns per free elem (fp32, 0.96 GHz)
    GPS_NS = 1.984                  # Pool TensorTensor add at 0.42 efficiency
    dve, gps = [taps[0]], [taps[1], taps[2]]   # seeds dy=9 (full coverage);
    td = 98000.0 + taps[0][4] * NB * DVE_NS * 2    # gps seed = add of 2 premults
    tg = 5000.0 + (taps[1][4] + taps[2][4]) * NB * GPS_NS
    for t in taps[3:]:
        cd = t[4] * NB * DVE_NS * 2        # 2 partition groups
        cg = t[4] * NB * GPS_NS * 2
        if td + cd <= tg + cg:
            dve.append(t)
            td += cd
        else:
            gps.append(t)
            tg += cg
    return dve, gps


def build_host_weights(inp):
    """Layout transforms / dtype splits of the weight inputs (host side)."""
    w = {}
    W1 = np.zeros((201, 200), np.float32)
    c1w = np.asarray(inp["c1w"]).reshape(25, 49)
    for c in range(25):
        for o in range(8):
            for t in range(49):
                i = o * 25 - 24 + t
                if 0 <= i < 200:
                    W1[i, c * 8 + o] = c1w[c, t]
    W1[200, :] = np.repeat(np.asarray(inp["c1b"]), 8)
    w["W1big"] = W1

    for name, wk, bk in [("W2big", "c2w", "c2b"), ("W3big", "c3w", "c3b")]:
        Wb = np.zeros((201, 200), np.float32)
        cw = np.asarray(inp[wk]).reshape(25, 25, 3)
        for co in range(25):
            for o in range(8):
                for ci in range(25):
                    for t in range(3):
                        oi = o + t - 1
                        if 0 <= oi < 8:
                            Wb[ci * 8 + oi, co * 8 + o] = 0.5 * cw[co, ci, t]
        Wb[200, :] = np.repeat(np.asarray(inp[bk]), 8)
        w[name] = Wb

    k = np.arange(101)[None, :]
    n = np.arange(200)[:, None]
    ang = -2.0 * np.pi * k * n / 200.0
    F = np.zeros((201, 202), np.float64)
    F[:200, :101] = np.cos(ang) / 200.0
    F[:200, 101:] = np.sin(ang) / 200.0
    w["Fcat"] = F.astype(np.float32)

    sw = np.zeros((102, 200), np.float32)
    sw[:101] = np.asarray(inp["spec_w"]).T
    sw[101] = np.asarray(inp["spec_b"])
    w["spec_wT"] = sw

    for i, (sk, bk) in enumerate([("gn1s", "gn1b"), ("gn2s", "gn2b"), ("gn3s", "gn3b")], 1):
        w[f"gn{i}gamma"] = np.repeat(np.asarray(inp[sk]), 8).astype(np.float32).reshape(200, 1)
        w[f"gn{i}beta"] = np.repeat(np.asarray(inp[bk]), 8).astype(np.float32).reshape(200, 1)

    gm = np.zeros((200, 5), np.float32)
    for p in range(200):
        gm[p, p // 40] = 1.0
    w["gmask"] = gm
    w["gmaskT"] = np.ascontiguousarray(gm.T)

    pw = np.asarray(inp["pos_w"]).reshape(200, 133).astype(np.float64)
    w["posw"] = pw.astype(np.float32)       # true weights (DVE B chain)
    w["posb"] = np.asarray(inp["pos_b"]).astype(np.float32).reshape(200, 1)
    wA = pw[:128]
    wA16 = wA.astype(np.float16)
    dAh = np.zeros((133, 128, 128), np.float16)
    dAl = np.zeros((133, 128, 128), np.float16)
    dAr = np.zeros((133, 128, 128), np.float16)
    for t in range(133):
        for k in range(128):
            dAh[t, k, k] = wA16[k, t]
            dAl[t, k, k] = np.float16(wA16[k, t].astype(np.float64) / 64.0)
            dAr[t, k, k] = np.float16(wA[k, t] - wA16[k, t].astype(np.float64))
    w["dgAh"] = dAh
    w["dgAl"] = dAl
    w["dgAr"] = dAr
    w["poswr"] = (wA - wA16.astype(np.float64)).astype(np.float32)
    for i, nm in enumerate(["W1big", "W2big", "W3big"], 1):
        w[f"cbias{i}"] = np.ascontiguousarray(w[nm][200]).reshape(200, 1)

    # ---- VQ tables (host-precomputed; pure weight transforms) ----
    iw = np.asarray(inp["inp_w"]).astype(np.float64)        # [LLM, DM]
    cb = np.asarray(inp["codebook"]).astype(np.float64)     # [KC, LLM]
    CB2 = iw.T @ cb.T                                        # [DM, KC]
    hi = CB2.astype(np.float16)
    lo = (CB2 - hi.astype(np.float64)).astype(np.float16)
    w["cb2hA"] = np.ascontiguousarray(hi[:128])
    w["cb2lA"] = np.ascontiguousarray(lo[:128])

    # norm rows: nvec2 = inp_b.c - 0.5|c|^2, 4-way fp16 split
    nvec2 = cb @ np.asarray(inp["inp_b"]).astype(np.float64) - 0.5 * (cb * cb).sum(-1)
    n1 = nvec2.astype(np.float16).astype(np.float64)
    r = nvec2 - n1
    n2 = r.astype(np.float16).astype(np.float64)
    r = r - n2
    n3 = r.astype(np.float16).astype(np.float64)
    n4 = r - n3
    # B tiles: rows 0..71 = dm 128..199, 72..95 = zeros, 96..97 = norm rows
    # (32-aligned partition base for the ones-rows memsets in pe16 tiles)
    hB = np.zeros((98, KC), np.float16)
    lB = np.zeros((98, KC), np.float16)
    hB[:72] = hi[128:200]
    lB[:72] = lo[128:200]
    hB[96], hB[97] = n1.astype(np.float16), n3.astype(np.float16)
    lB[96], lB[97] = n2.astype(np.float16), n4.astype(np.float16)
    w["cb2hB"] = hB
    w["cb2lB"] = lB

    w2 = cb @ np.asarray(inp["outp_w"]).astype(np.float64).T \
        + np.asarray(inp["outp_b"]).astype(np.float64)
    w["w2f"] = w2.astype(np.float16)                         # [KC, DM]
    return w


def _build_nc(debug=False):
    from contextlib import ExitStack
    import concourse.bass as bass
    import concourse.mybir as mybir
    import concourse.tile as tile
    from concourse import bacc

    f32 = mybir.dt.float32
    f16 = mybir.dt.float16
    u32 = mybir.dt.uint32
    Alu = mybir.AluOpType
    AF = mybir.ActivationFunctionType
    AX = mybir.AxisListType.X

    nc = bacc.Bacc("TRN2", target_bir_lowering=False, debug=False, num_devices=NCORES)

    di = {}
    di["xT"] = nc.dram_tensor("xT", [200, TOK], f32, kind="ExternalInput")
    for nm in ["W1big", "W2big", "W3big"]:
        di[nm] = nc.dram_tensor(nm, [201, 200], f32, kind="ExternalInput")
    di["Fcat"] = nc.dram_tensor("Fcat", [201, 202], f32, kind="ExternalInput")
    di["spec_wT"] = nc.dram_tensor("spec_wT", [102, 200], f32, kind="ExternalInput")
    for i in range(1, 4):
        di[f"gn{i}gamma"] = nc.dram_tensor(f"gn{i}gamma", [200, 1], f32, kind="ExternalInput")
        di[f"gn{i}beta"] = nc.dram_tensor(f"gn{i}beta", [200, 1], f32, kind="ExternalInput")
    di["gmask"] = nc.dram_tensor("gmask", [200, 5], f32, kind="ExternalInput")
    di["gmaskT"] = nc.dram_tensor("gmaskT", [5, 200], f32, kind="ExternalInput")
    di["posw"] = nc.dram_tensor("posw", [200, 133], f32, kind="ExternalInput")
    for nm in ["dgAh", "dgAl", "dgAr"]:
        di[nm] = nc.dram_tensor(nm, [133, 128, 128], f16, kind="ExternalInput")
    di["poswr"] = nc.dram_tensor("poswr", [128, 133], f32, kind="ExternalInput")
    for i in range(1, 4):
        di[f"cbias{i}"] = nc.dram_tensor(f"cbias{i}", [200, 1], f32, kind="ExternalInput")
    di["posb"] = nc.dram_tensor("posb", [200, 1], f32, kind="ExternalInput")
    di["cb2hA"] = nc.dram_tensor("cb2hA", [128, KC], f16, kind="ExternalInput")
    di["cb2lA"] = nc.dram_tensor("cb2lA", [128, KC], f16, kind="ExternalInput")
    di["cb2hB"] = nc.dram_tensor("cb2hB", [98, KC], f16, kind="ExternalInput")
    di["cb2lB"] = nc.dram_tensor("cb2lB", [98, KC], f16, kind="ExternalInput")
    di["w2f"] = nc.dram_tensor("w2f", [KC, DM], f16, kind="ExternalInput")

    out_d = nc.dram_tensor("out", [TOK, 200], f16, kind="ExternalOutput")
    idx_d = nc.dram_tensor("idx", [128, 18], u32, kind="ExternalOutput")
    dbg = {}
    if debug:
        for nm in ["d_pe", "d_g1", "d_pe1"]:
            dbg[nm] = nc.dram_tensor(nm, [200, TOK], f32, kind="ExternalOutput")

    TT = _tok_tiles()
    NS = _n_slices()
    PSH = [128, NB * PBLK]     # padded_shape for pos-conv-sized fe2 tags

    with tile.TileContext(nc) as tc:
        late = ExitStack()
        with late, (
            tc.tile_pool(name="persist", bufs=1)) as persist, (
            tc.tile_pool(name="pconst", bufs=1)) as pconst, (
            tc.tile_pool(name="mid", bufs=1)) as mid:
            gidxu = persist.tile([128, 18], u32, name="gidxu")
            onesT = pconst.tile([1, 512], f32, name="onesT")
            nc.vector.memset(onesT[:], 1.0)
            z16 = pconst.tile([1, 128], f16, name="z16")
            nc.vector.memset(z16[:], 0.0)
            ones512h = pconst.tile([1, 512], f16, name="ones512h")
            nc.vector.memset(ones512h[:], 1.0)

            # score tables (host-precomputed), loaded once
            cb2hA = mid.tile([128, KC], f16, name="cb2hA")
            cb2lA = mid.tile([128, KC], f16, name="cb2lA")
            cb2hB = mid.tile([98, KC], f16, name="cb2hB")
            cb2lB = mid.tile([98, KC], f16, name="cb2lB")
            nc.sync.dma_start(cb2hA[:], di["cb2hA"][:])
            nc.sync.dma_start(cb2lA[:], di["cb2lA"][:])
            nc.sync.dma_start(cb2hB[:], di["cb2hB"][:])
            nc.sync.dma_start(cb2lB[:], di["cb2lB"][:])

            # ---------------- Front end ----------------
            with (
                tc.tile_pool(name="fe2", bufs=1) as fe2,
                tc.tile_pool(name="fetmp", bufs=2) as fetmp,
            ):
                gmA = pconst.tile([128, 5], f32, name="gmA")
                gmB = pconst.tile([72, 5], f32, name="gmB")
                gmT = pconst.tile([5, 200], f32, name="gmT")
                nc.sync.dma_start(gmA[:], di["gmask"][0:128, :])
                nc.sync.dma_start(gmB[:], di["gmask"][128:200, :])
                nc.sync.dma_start(gmT[:], di["gmaskT"][:])

                g1A = fe2.tile([128, TOK], f32, name="g1A", tag="gA1", padded_shape=PSH)
                g1B = fe2.tile([72, TOK], f32, name="g1B", tag="gB1", padded_shape=PSH)
                g2A = fe2.tile([128, TOK], f32, name="g2A", tag="gA2")
                g2B = fe2.tile([72, TOK], f32, name="g2B", tag="gB2")
                g3A = fe2.tile([128, TOK], f32, name="g3A", tag="gA1", padded_shape=PSH)
                g3B = fe2.tile([72, TOK], f32, name="g3B", tag="gB1", padded_shape=PSH)
                pe1A = fe2.tile([128, TOK], f32, name="pe1A", tag="gA2")
                pe1B = fe2.tile([72, TOK], f32, name="pe1B", tag="gB2")

                def conv_gn_gelu(fe1, feps, stps, rhsA, rhsB, wname, gi, outA, outB,
                                 dbg_g=None):
                    """rhs [128,TOK]/[72,TOK] + onesT -> g = 2*gelu(GN(conv))."""
                    WA = fetmp.tile([128, 200], f32, name=f"WA{gi}", tag="WA")
                    WB = fetmp.tile([72, 200], f32, name=f"WB{gi}", tag="WB")
                    cbA = fetmp.tile([128, 1], f32, name=f"cbA{gi}", tag="cbA")
                    cbB = fetmp.tile([72, 1], f32, name=f"cbB{gi}", tag="cbB")
                    nc.sync.dma_start(WA[:], di[wname][0:128, :])
                    nc.sync.dma_start(WB[:], di[wname][128:200, :])
                    nc.sync.dma_start(cbA[:], di[f"cbias{gi}"][0:128, :])
                    nc.sync.dma_start(cbB[:], di[f"cbias{gi}"][128:200, :])
                    gam = fetmp.tile([128, 2], f32, name=f"gam{gi}", tag="gam")
                    bet = fetmp.tile([128, 2], f32, name=f"bet{gi}", tag="bet")
                    nc.sync.dma_start(gam[0:128, 0:1], di[f"gn{gi}gamma"][0:128, :])
                    nc.sync.dma_start(gam[0:72, 1:2], di[f"gn{gi}gamma"][128:200, :])
                    nc.sync.dma_start(bet[0:128, 0:1], di[f"gn{gi}beta"][0:128, :])
                    nc.sync.dma_start(bet[0:72, 1:2], di[f"gn{gi}beta"][128:200, :])

                    convA = fe1.tile([128, TOK], f32, name=f"convA{gi}", tag="convA")
                    convB = fe1.tile([72, TOK], f32, name=f"convB{gi}", tag="convB")
                    for (m0, msz, cdst, cbv) in [(0, 128, convA, cbA), (128, 72, convB, cbB)]:
                        for (n0, nsz) in NS:
                            cps = feps.tile([128, 512], f32, name="cps", tag="cps")
                            nc.tensor.matmul(cps[:msz, :nsz], WA[:, m0:m0 + msz],
                                             rhsA[:, n0:n0 + nsz], start=True, stop=False)
                            nc.tensor.matmul(cps[:msz, :nsz], WB[:, m0:m0 + msz],
                                             rhsB[:, n0:n0 + nsz], start=False, stop=True)
                            nc.scalar.activation(cdst[:, n0:n0 + nsz], cps[:msz, :nsz],
                                                 AF.Identity, bias=cbv[:msz, 0:1])

                    stA = fetmp.tile([128, 8], f32, name=f"stA{gi}", tag="stA")
                    stB = fetmp.tile([72, 8], f32, name=f"stB{gi}", tag="stB")
                    sqA = fe2.tile([128, TOK], f32, name=f"sqA{gi}", tag="sqA", padded_shape=PSH)
                    sqB = fe2.tile([72, TOK], f32, name=f"sqB{gi}", tag="sqB", padded_shape=PSH)
                    nc.scalar.square(sqA[:], convA[:])
                    nc.scalar.square(sqB[:], convB[:])
                    for b in range(NB):
                        sl = slice(b * T1, (b + 1) * T1)
                        nc.vector.reduce_sum(stA[:, 2 * b:2 * b + 1], convA[:, sl], axis=AX)
                        nc.vector.reduce_sum(stA[:, 2 * b + 1:2 * b + 2], sqA[:, sl], axis=AX)
                        nc.vector.reduce_sum(stB[:, 2 * b:2 * b + 1], convB[:, sl], axis=AX)
                        nc.vector.reduce_sum(stB[:, 2 * b + 1:2 * b + 2], sqB[:, sl], axis=AX)
                    sps = stps.tile([5, 8], f32, name="sps", tag="stp")
                    nc.tensor.matmul(sps[:], gmA[:], stA[:], start=True, stop=False)
                    nc.tensor.matmul(sps[:], gmB[:], stB[:], start=False, stop=True)

                    st = fetmp.tile([5, 16], f32, name=f"st{gi}", tag="st")
                    st2 = fetmp.tile([5, 8], f32, name=f"st2{gi}", tag="st2")
                    NINV = 1.0 / (40 * T1)
                    nc.vector.tensor_scalar(st[:, 0:8], sps[:], NINV, None, op0=Alu.mult)
                    for b in range(NB):
                        nc.vector.tensor_copy(st2[:, b:b + 1], st[:, 2 * b:2 * b + 1])
                        nc.vector.tensor_mul(st[:, 8 + b:9 + b], st[:, 2 * b:2 * b + 1],
                                             st[:, 2 * b:2 * b + 1])
                        nc.vector.tensor_sub(st2[:, 4 + b:5 + b], st[:, 2 * b + 1:2 * b + 2],
                                             st[:, 8 + b:9 + b])
                    nc.vector.tensor_scalar(st2[:, 4:8], st2[:, 4:8], EPS, None, op0=Alu.add)
                    sqr = fetmp.tile([5, 4], f32, name=f"sqr{gi}", tag="sqr")
                    nc.scalar.activation(sqr[:], st2[:, 4:8], AF.Sqrt)
                    r0 = fetmp.tile([5, 4], f32, name=f"r0{gi}", tag="r0")
                    nc.vector.reciprocal(r0[:], sqr[:])
                    tn = fetmp.tile([5, 4], f32, name=f"tn{gi}", tag="tn")
                    nc.vector.tensor_mul(tn[:], r0[:], r0[:])
                    nc.vector.tensor_mul(tn[:], tn[:], st2[:, 4:8])
                    nc.vector.tensor_scalar(tn[:], tn[:], -0.5, 1.5, op0=Alu.mult, op1=Alu.add)
                    nc.vector.tensor_mul(st2[:, 4:8], r0[:], tn[:])

                    bpsA = stps.tile([128, 8], f32, name="bpsA", tag="stp")
                    bpsB = stps.tile([72, 8], f32, name="bpsB", tag="stp")
                    nc.tensor.matmul(bpsA[:], gmT[:, 0:128], st2[:], start=True, stop=True)
                    nc.tensor.matmul(bpsB[:], gmT[:, 128:200], st2[:], start=True, stop=True)
                    rgA = fetmp.tile([128, 8], f32, name=f"rgA{gi}", tag="rgA")
                    rgB = fetmp.tile([72, 8], f32, name=f"rgB{gi}", tag="rgB")
                    for (bps, rg, gcol, prt) in [(bpsA, rgA, 0, 128), (bpsB, rgB, 1, 72)]:
                        nc.vector.tensor_scalar(rg[:prt, 0:4], bps[:prt, 4:8],
                                                gam[:prt, gcol:gcol + 1], None, op0=Alu.mult)
                        nc.vector.tensor_mul(rg[:prt, 4:8], bps[:prt, 0:4], rg[:prt, 0:4])
                        nc.vector.tensor_scalar(rg[:prt, 4:8], rg[:prt, 4:8],
                                                bet[:prt, gcol:gcol + 1], None, op0=Alu.subtract)
                    zA = fe2.tile([128, TOK], f32, name=f"zA{gi}", tag="zA", padded_shape=PSH)
                    zB = fe2.tile([72, TOK], f32, name=f"zB{gi}", tag="zB", padded_shape=PSH)
                    for b in range(NB):
                        sl = slice(b * T1, (b + 1) * T1)
                        nc.vector.tensor_scalar(zA[:, sl], convA[:, sl], rgA[:, b:b + 1],
                                                rgA[:, 4 + b:5 + b], op0=Alu.mult, op1=Alu.subtract)
                        nc.vector.tensor_scalar(zB[:, sl], convB[:, sl], rgB[:, b:b + 1],
                                                rgB[:, 4 + b:5 + b], op0=Alu.mult, op1=Alu.subtract)
                    eA = fe1.tile([128, TOK], f32, name=f"eA{gi}", tag="convA")
                    eB = fe1.tile([72, TOK], f32, name=f"eB{gi}", tag="convB")
                    nc.scalar.activation(eA[:], zA[:], AF.Erf, scale=SQ2I)
                    nc.scalar.activation(eB[:], zB[:], AF.Erf, scale=SQ2I)
                    nc.vector.scalar_tensor_tensor(outA[0:128, :], eA[:], 1.0, zA[:],
                                                   op0=Alu.add, op1=Alu.mult)
                    nc.vector.scalar_tensor_tensor(outB[0:72, :], eB[:], 1.0, zB[:],
                                                   op0=Alu.add, op1=Alu.mult)
                    if dbg_g is not None:
                        nc.sync.dma_start(dbg_g[0:128, :], outA[0:128, :])
                        nc.sync.dma_start(dbg_g[128:200, :], outB[0:72, :])

                with (
                    tc.tile_pool(name="fe1", bufs=1) as fe1,
                    tc.tile_pool(name="feps", bufs=3, space="PSUM") as feps,
                    tc.tile_pool(name="stps", bufs=1, space="PSUM") as stps,
                ):
                    # --- x arrives pre-transposed: [200, TOK]
                    xA = fe1.tile([128, TOK], f32, name="xA", tag="xA")
                    xB = fe1.tile([72, TOK], f32, name="xB", tag="xB")
                    nc.sync.dma_start(xA[:], di["xT"][0:128, :])
                    nc.sync.dma_start(xB[:], di["xT"][128:200, :])

                    conv_gn_gelu(fe1, feps, stps, xA, xB, "W1big", 1, g1A, g1B,
                                 dbg.get("d_g1"))
                    conv_gn_gelu(fe1, feps, stps, g1A, g1B, "W2big", 2, g2A, g2B)
                    conv_gn_gelu(fe1, feps, stps, g2A, g2B, "W3big", 3, g3A, g3B)

                    # --- FFT + spec proj; pe1 = 0.5*g3 + specproj
                    FA = fetmp.tile([128, 202], f32, name="FA", tag="WA")
                    FB = fetmp.tile([72, 202], f32, name="FB", tag="WB")
                    nc.sync.dma_start(FA[:], di["Fcat"][0:128, :])
                    nc.sync.dma_start(FB[:], di["Fcat"][128:200, :])
                    reT = fe2.tile([101, TOK], f32, name="reT", tag="sqA", padded_shape=PSH)
                    imT = fe2.tile([101, TOK], f32, name="imT", tag="sqB", padded_shape=PSH)
                    for (m0, dst) in [(0, reT), (101, imT)]:
                        for (n0, nsz) in NS:
                            cps = feps.tile([128, 512], f32, name="cpsf", tag="cps")
                            nc.tensor.matmul(cps[:101, :nsz], FA[:, m0:m0 + 101],
                                             xA[:, n0:n0 + nsz], start=True, stop=False)
                            nc.tensor.matmul(cps[:101, :nsz], FB[:, m0:m0 + 101],
                                             xB[:, n0:n0 + nsz], start=False, stop=True)
                            nc.scalar.activation(dst[:, n0:n0 + nsz], cps[:101, :nsz], AF.Copy)
                    specA = fe1.tile([101, TOK], f32, name="specA", tag="convA")
                    nc.vector.tensor_mul(reT[:], reT[:], reT[:])
                    nc.vector.tensor_mul(imT[:], imT[:], imT[:])
                    nc.vector.tensor_add(reT[:], reT[:], imT[:])
                    epsb = fetmp.tile([101, 1], f32, name="epsb", tag="gam")
                    nc.vector.memset(epsb[:], 1e-30)
                    nc.scalar.activation(specA[0:101, :], reT[:], AF.Sqrt, bias=epsb[:])
                    swT = fetmp.tile([101, 200], f32, name="swT", tag="WB")
                    swTb = fetmp.tile([1, 200], f32, name="swTb", tag="Wb")
                    nc.sync.dma_start(swT[:], di["spec_wT"][0:101, :])
                    nc.sync.dma_start(swTb[:], di["spec_wT"][101:102, :])
                    for (m0, msz, gsrc, pdst) in [(0, 128, g3A, pe1A), (128, 72, g3B, pe1B)]:
                        for (n0, nsz) in NS:
                            cps = feps.tile([128, 512], f32, name="cpss", tag="cps")
                            nc.tensor.matmul(cps[:msz, :nsz], swT[:, m0:m0 + msz],
                                             specA[:, n0:n0 + nsz], start=True, stop=False)
                            nc.tensor.matmul(cps[:msz, :nsz], swTb[:, m0:m0 + msz],
                                             onesT[:, 0:nsz], start=False, stop=True)
                            nc.vector.scalar_tensor_tensor(
                                pdst[:, n0:n0 + nsz], gsrc[:msz, n0:n0 + nsz], 0.5,
                                cps[:msz, :nsz], op0=Alu.mult, op1=Alu.add)
                    if debug:
                        nc.sync.dma_start(dbg["d_pe1"][0:128, :], pe1A[:])
                        nc.sync.dma_start(dbg["d_pe1"][128:200, :], pe1B[:])

                # pe'' tiles: pool opened after fe1 freed its space,
                # closed (via `late`) after the scores phase.
                pe16 = late.enter_context(tc.tile_pool(name="pe16", bufs=1, side="right"))
                pehA = pe16.tile([128, TOK], f16, name="pehA")
                pelA = pe16.tile([128, TOK], f16, name="pelA")
                pehB = pe16.tile([98, TOK], f16, name="pehB")
                pelB = pe16.tile([98, TOK], f16, name="pelB")
                peA = pe16.tile([128, TOK], f32, name="peA")
                peB = pe16.tile([72, TOK], f32, name="peB")
                nc.vector.memset(pehB[64:98, :], 0.0)
                nc.vector.memset(pehB[96:98, :], 1.0)
                nc.vector.memset(pelB[64:98, :], 0.0)

                # --- pos conv: A group (ch 0..127) on the PE as per-tap
                # diagonal f16 matmuls (K=M=128 -> fast-weight-load) with fp32
                # PSUM accumulation, three passes: hi (w16 x pad_hi), lo
                # (w16/64 x 64*pad_lo), wres ((w-w16) denormal-f16 x pad_hi)
                # -> ~1e-6 exact. B group (ch 128..199) as an exact fp32 DVE
                # MAC chain with true weights. Both run concurrently.
                pwB = fetmp.tile([72, 133], f32, name="pwB", tag="pwB")
                nc.sync.dma_start(pwB[:], di["posw"][128:200, :])
                pwAr = fetmp.tile([128, 133], f32, name="pwAr", tag="WA")
                nc.sync.dma_start(pwAr[:], di["poswr"][0:128, :])
                pbA = fetmp.tile([128, 1], f32, name="pbA", tag="cbA")
                pbB = fetmp.tile([72, 1], f32, name="pbB", tag="cbB")
                nc.sync.dma_start(pbA[:], di["posb"][0:128, :])
                nc.sync.dma_start(pbB[:], di["posb"][128:200, :])

                padhA = fe2.tile([128, NB, PBLK], f16, name="padhA", tag="zA")
                padlA = fe2.tile([128, NB, PBLK], f16, name="padlA", tag="sqA")
                padB = fe2.tile([72, NB, PBLK], f32, name="padB", tag="zB")
                accB = fe2.tile([72, NB, PBLK], f32, name="accB", tag="gB1")
                accWr = fe2.tile([128, NB, PBLK], f32, name="accWr", tag="gA1")
                h16A = fe2.tile([128, TOK], f16, name="h16A", tag="h16A")
                lo16A = fe2.tile([128, TOK], f16, name="lo16A", tag="lo16A")
                lo32 = fe2.tile([128, TOK], f32, name="lo32", tag="gA1")
                nc.vector.memset(padhA[:], 0.0)
                nc.vector.memset(padlA[:], 0.0)
                nc.vector.memset(padB[:], 0.0)
                nc.scalar.activation(h16A[:], pe1A[:], AF.Copy)
                nc.vector.tensor_sub(lo32[:], pe1A[:], h16A[:])
                nc.vector.tensor_scalar(lo16A[:], lo32[:], 64.0, None, op0=Alu.mult)
                for b in range(NB):
                    for (ptile, flat, prt) in [(padhA, h16A, 128), (padlA, lo16A, 128),
                                               (padB, pe1B, 72)]:
                        dst = ptile[:, b, 4:688].rearrange("p (h w) -> p h w", w=36)
                        nc.scalar.activation(
                            dst[:, :, 3:33],
                            flat[:prt, b * T1:(b + 1) * T1]
                            .rearrange("p (h w) -> p h w", w=30),
                            AF.Copy)

                # B: DVE fp32 MAC chain (first tap overwrites, no acc memset)
                for i, (dy, dx, sb, db, L) in enumerate(_pos_taps()):
                    tap = dy * 7 + dx
                    srcw = padB[:, :, sb:sb + L]
                    dst = accB[:, :, db:db + L]
                    if i == 0:
                        nc.vector.tensor_scalar(dst, srcw, pwB[:, tap:tap + 1], None,
                                                op0=Alu.mult)
                    else:
                        nc.vector.scalar_tensor_tensor(dst, srcw, pwB[:, tap:tap + 1],
                                                       dst, op0=Alu.mult, op1=Alu.add)
                # A wres tail (last taps) on DVE: fills DVE slack, shrinks the
                # PE wres pass. First subset tap is dy=9 (full acc coverage).
                _all = _pos_taps()
                wres_dve = [_all[6]] + _all[83:]
                wres_pe = [t for t in _all if t not in wres_dve]
                for i, (dy, dx, sb, db, L) in enumerate(wres_dve):
                    tap = dy * 7 + dx
                    srcw = padhA[:, :, sb:sb + L]
                    dst = accWr[:, :, db:db + L]
                    if i == 0:
                        nc.vector.tensor_scalar(dst, srcw, pwAr[:, tap:tap + 1], None,
                                                op0=Alu.mult)
                    else:
                        nc.vector.scalar_tensor_tensor(dst, srcw, pwAr[:, tap:tap + 1],
                                                       dst, op0=Alu.mult, op1=Alu.add)

                # A: PE passes
                def bank_chunks(d0, L):
                    out, c = [], d0
                    while c < d0 + L:
                        e = min((c // 512 + 1) * 512, d0 + L)
                        out.append((c, e - c))
                        c = e
                    return out

                with (
                    tc.tile_pool(name="posps", bufs=1, space="PSUM") as posps,
                    tc.tile_pool(name="dgp", bufs=4) as dgp,
                ):
                    pps = posps.tile([128, 2736], f32, name="pps")
                    for c in range(0, 2736, 512):
                        n = min(512, 2736 - c)
                        nc.tensor.matmul(pps[:128, c:c + n], z16[:, 0:128],
                                         ones512h[:, 0:n], start=True, stop=False)
                    for (dgd, rhs, tlist) in [(di["dgAh"], padhA, _all),
                                              (di["dgAl"], padlA, _all),
                                              (di["dgAr"], padhA, wres_pe)]:
                        for (dy, dx, sbg, dbg_, L) in tlist:
                            d = dy - 9
                            ho, hn, hs = max(0, -d), 19 - abs(d), max(0, d)
                            shift = (hs - ho) * 36 + dx - 3
                            dg = dgp.tile([128, 128], f16, name="dg", tag="dg")
                            nc.sync.dma_start(dg[:], dgd[dy * 7 + dx, :, :])
                            for b in range(NB):
                                d0 = b * 684 + ho * 36
                                for (c, n) in bank_chunks(d0, hn * 36):
                                    sb0 = 4 + (c - b * 684) + shift
                                    nc.tensor.matmul(
                                        pps[:128, c:c + n], dg[:, :],
                                        rhs[:, b, sb0:sb0 + n],
                                        start=False, stop=False)
                    for c in range(0, 2736, 512):
                        n = min(512, 2736 - c)
                        nc.tensor.matmul(pps[:128, c:c + n], z16[:, 0:128],
                                         ones512h[:, 0:n], start=False, stop=True)
                    # assemblies: pe'' = (acc + posb) + pe1; then f16 hi/lo split
                    for (prt, accv_fn, wr, pb, src_, pe, peh, pel) in [
                            (72, lambda b: accB[:, b, 4:688]
                             .rearrange("p (h w) -> p h w", w=36), None, pbB, pe1B,
                             peB, pehB, pelB),
                            (128, lambda b: pps[:128, b * 684:(b + 1) * 684]
                             .rearrange("p (h w) -> p h w", w=36), accWr, pbA, pe1A,
                             peA, pehA, pelA)]:
                        for b in range(NB):
                            ov = pe[:prt, b * T1:(b + 1) * T1] \
                                .rearrange("p (h w) -> p h w", w=30)
                            nc.vector.scalar_tensor_tensor(
                                ov, accv_fn(b)[:, :, 3:33], pb[:, 0:1],
                                src_[:prt, b * T1:(b + 1) * T1]
                                .rearrange("p (h w) -> p h w", w=30),
                                op0=Alu.add, op1=Alu.add)
                            if wr is not None:
                                wv = wr[:, b, 4:688].rearrange(
                                    "p (h w) -> p h w", w=36)
                                nc.vector.tensor_add(ov, ov, wv[:, :, 3:33])
                        nc.scalar.activation(peh[:prt, :], pe[:prt, :], AF.Copy)
                        nc.vector.tensor_sub(pel[:prt, :], pe[:prt, :], peh[:prt, :])
                if debug:
                    nc.sync.dma_start(dbg["d_pe"][0:128, :], peA[:])
                    nc.sync.dma_start(dbg["d_pe"][128:200, :], peB[:])

            # ------- scores (tok-tile outer) + argmax + indirect gather
            with (
                tc.tile_pool(name="sce", bufs=2) as sce,
                tc.tile_pool(name="gat", bufs=3) as gat,
                tc.tile_pool(name="scps", bufs=4, space="PSUM") as scps,
            ):
                for ti, (t0, tsz) in enumerate(TT):
                    tsl = slice(t0, t0 + tsz)
                    sc = sce.tile([128, KC], f32, name="sc", tag="sc")
                    for kc in range(8):
                        csl = slice(kc * 512, (kc + 1) * 512)
                        sps_ = scps.tile([128, 512], f32, name="sps_", tag="sps")
                        seq = [
                            (pehA, cb2hA), (pehB, cb2hB),   # term1 (+norm hi)
                            (pelA, cb2hA), (pelB, cb2hB),   # term2
                            (pehA, cb2lA), (pehB, cb2lB),   # term3 (+norm lo)
                        ]
                        for i, (lh, rh) in enumerate(seq):
                            nc.tensor.matmul(sps_[:tsz, :], lh[:, tsl], rh[:, csl],
                                             start=(i == 0), stop=(i == len(seq) - 1))
                        nc.scalar.activation(sc[:tsz, csl], sps_[:tsz, :], AF.Copy)
                    mv8 = gat.tile([128, 8], f32, name="mv8", tag="mv8")
                    mi8 = gat.tile([128, 8], u32, name="mi8", tag="mi8")
                    nc.vector.max_with_indices(mv8[:tsz, :], mi8[:tsz, :], sc[:tsz, :])
                    nc.vector.tensor_copy(gidxu[:tsz, ti:ti + 1], mi8[:tsz, 0:1])
                    grow = gat.tile([128, 200], f16, name="grow", tag="grow")
                    nc.gpsimd.indirect_dma_start(
                        out=grow[:tsz, :], out_offset=None,
                        in_=di["w2f"][:],
                        in_offset=bass.IndirectOffsetOnAxis(ap=mi8[:tsz, 0:1], axis=0))
                    nc.sync.dma_start(out_d[t0:t0 + tsz, :], grow[:tsz, :])
                nc.sync.dma_start(idx_d[:], gidxu[:])

    nc.compile()
    return nc


def _prep_inputs(inp):
    w = build_host_weights(inp)
    x = np.asarray(inp["x"], np.float32).reshape(B * T1, 200)
    in_maps = []
    for c in range(NCORES):
        m = {"xT": np.ascontiguousarray(x[c * TOK:(c + 1) * TOK].T)}
        for k in ["W1big", "W2big", "W3big", "Fcat", "spec_wT", "gmask", "gmaskT",
                  "posw", "posb", "poswr", "dgAh", "dgAl", "dgAr", "cb2hA", "cb2lA", "cb2hB", "cb2lB", "w2f",
                  "cbias1", "cbias2", "cbias3"]:
            m[k] = np.ascontiguousarray(w[k])
        for i in range(1, 4):
            m[f"gn{i}gamma"] = np.ascontiguousarray(w[f"gn{i}gamma"])
            m[f"gn{i}beta"] = np.ascontiguousarray(w[f"gn{i}beta"])
        in_maps.append(m)
    return in_maps


def run(inp, debug=False, trace=False, **kw):
    global _COMPILED
    from concourse.bass_utils import run_bass_kernel_spmd
    if _COMPILED is None or _COMPILED[1] != debug:
        _COMPILED = (_build_nc(debug=debug), debug)
    nc = _COMPILED[0]
    in_maps = _prep_inputs(inp)
    res = run_bass_kernel_spmd(nc, in_maps, core_ids=list(range(NCORES)), trace=trace, **kw)
    return res


def kernel(**inputs):
    res = run(inputs)
    out = np.concatenate([np.asarray(r["out"], np.float32) for r in res.results], 0)
    return out.reshape(B, CH, NP_, DM)


# revision 23
# speedup vs baseline: 1.0424x; 1.0424x over previous
"""Trainium2 Bass kernel for nn_CSBrainLLMVQ (CSBrain conv front-end + LLM VQ codebook).

Sharding: data-parallel over batch (4 batches/core x 8 cores). Per core:
  conv chain / GroupNorm / GELU(erf) / rFFT / depthwise pos-conv in fp32
  (feature-on-partition, token-on-free layout; convs as fp32 matmuls).
  The VQ reduction CB2T[dm,c] = sum_llm inp_w[llm,dm]*cb[c,llm] and the
  output table W2f[c,:] = cb[c] @ outp_w.T + outp_b are precomputed on the
  host (pure weight transforms), so the device only runs the front-end and
  the [tok,200]x[200,4096] score contraction. Scores use hi/lo fp16
  compensation (3 terms x 2 partition groups = 6 matmuls per 512-col chunk),
  keeping the fp32-grade argmin exact. The 133-tap depthwise positional conv
  splits by channel group: ch 0..127 run on the TensorEngine as per-tap
  diagonal f16 matmuls (K=M=128, fast-weight-load) accumulating in fp32 PSUM
  across three compensated passes (hi, scaled-lo, denormal w-residual,
  ~1e-6 exact); ch 128..199 run concurrently as an exact fp32 DVE MAC chain
  over a flat 36-wide padded layout. Argmin via one fp32 max8/find_index8
  pass per 128-token tile; output rows are fetched with indirect-DMA gathers
  from the host-precomputed W2f table and written out in f16 (upcast on host).
"""
import numpy as np

B, CH, NP_, PS = 32, 19, 30, 200
DM, LLM, KC = 200, 4096, 4096
EPS = 1e-5
T1 = CH * NP_          # 570 tokens per batch
NB = 4                 # batches per core
TOK = NB * T1          # 2280 tokens per core
NCORES = 8
SQ2I = 0.7071067811865476
PBLK = 724             # pos-conv per-batch block: 4 gutter + 19*36 + 36 zero row

_COMPILED = None


def _tok_tiles():
    out, t0 = [], 0
    while t0 < TOK:
        out.append((t0, min(128, TOK - t0)))
        t0 += 128
    return out


def _n_slices(width=512):
    out, n0 = [], 0
    while n0 < TOK:
        out.append((n0, min(width, TOK - n0)))
        n0 += width
    return out


def _pos_taps():
    """(dy, dx, src_base, dst_base, length) for each of the 133 taps, with a
    full-coverage dy=9 tap first for each engine (overwrite, no memset)."""
    taps = []
    order = [(9, dx) for dx in range(7)] + \
        [(dy, dx) for dy in range(19) if dy != 9 for dx in range(7)]
    for dy, dx in order:
        d = dy - 9
        ho, hn, hs = max(0, -d), 19 - abs(d), max(0, d)
        taps.append((dy, dx, 4 + hs * 36 + dx - 3, 4 + ho * 36, hn * 36))
    return taps


def _split_taps():
    """Greedy split balancing projected engine-busy time. DVE runs taps as
    native STT MACs; the GpSimd path runs them as ACT-premultiplied
    tensor-tensor adds (Pool ucode only supports TensorTensor)."""
    taps = _pos_taps()
    DVE_NS = 1.042                  # ns per free elem (fp32, 0.96 GHz)
    GPS_NS = 1.984                  # Pool TensorTensor add at 0.42 efficiency
    dve, gps = [taps[0]], [taps[1], taps[2]]   # seeds dy=9 (full coverage);
    td = 98000.0 + taps[0][4] * NB * DVE_NS * 2    # gps seed = add of 2 premults
    tg = 5000.0 + (taps[1][4] + taps[2][4]) * NB * GPS_NS
    for t in taps[3:]:
        cd = t[4] * NB * DVE_NS * 2        # 2 partition groups
        cg = t[4] * NB * GPS_NS * 2
        if td + cd <= tg + cg:
            dve.append(t)
            td += cd
        else:
            gps.append(t)
            tg += cg
    return dve, gps


def build_host_weights(inp):
    """Layout transforms / dtype splits of the weight inputs (host side)."""
    w = {}
    W1 = np.zeros((201, 200), np.float32)
    c1w = np.asarray(inp["c1w"]).reshape(25, 49)
    for c in range(25):
        for o in range(8):
            for t in range(49):
                i = o * 25 - 24 + t
                if 0 <= i < 200:
                    W1[i, c * 8 + o] = c1w[c, t]
    W1[200, :] = np.repeat(np.asarray(inp["c1b"]), 8)
    w["W1big"] = W1

    for name, wk, bk in [("W2big", "c2w", "c2b"), ("W3big", "c3w", "c3b")]:
        Wb = np.zeros((201, 200), np.float32)
        cw = np.asarray(inp[wk]).reshape(25, 25, 3)
        for co in range(25):
            for o in range(8):
                for ci in range(25):
                    for t in range(3):
                        oi = o + t - 1
                        if 0 <= oi < 8:
                            Wb[ci * 8 + oi, co * 8 + o] = 0.5 * cw[co, ci, t]
        Wb[200, :] = np.repeat(np.asarray(inp[bk]), 8)
        w[name] = Wb

    k = np.arange(101)[None, :]
    n = np.arange(200)[:, None]
    ang = -2.0 * np.pi * k * n / 200.0
    F = np.zeros((201, 202), np.float64)
    F[:200, :101] = np.cos(ang) / 200.0
    F[:200, 101:] = np.sin(ang) / 200.0
    w["Fcat"] = F.astype(np.float32)

    sw = np.zeros((102, 200), np.float32)
    sw[:101] = np.asarray(inp["spec_w"]).T
    sw[101] = np.asarray(inp["spec_b"])
    w["spec_wT"] = sw

    for i, (sk, bk) in enumerate([("gn1s", "gn1b"), ("gn2s", "gn2b"), ("gn3s", "gn3b")], 1):
        w[f"gn{i}gamma"] = np.repeat(np.asarray(inp[sk]), 8).astype(np.float32).reshape(200, 1)
        w[f"gn{i}beta"] = np.repeat(np.asarray(inp[bk]), 8).astype(np.float32).reshape(200, 1)

    gm = np.zeros((200, 5), np.float32)
    for p in range(200):
        gm[p, p // 40] = 1.0
    w["gmask"] = gm
    w["gmaskT"] = np.ascontiguousarray(gm.T)

    pw = np.asarray(inp["pos_w"]).reshape(200, 133).astype(np.float64)
    w["posw"] = pw.astype(np.float32)       # true weights (DVE B chain)
    w["posb"] = np.asarray(inp["pos_b"]).astype(np.float32).reshape(200, 1)
    wA = pw[:128]
    wA16 = wA.astype(np.float16)
    dAh = np.zeros((133, 128, 128), np.float16)
    dAl = np.zeros((133, 128, 128), np.float16)
    dAr = np.zeros((133, 128, 128), np.float16)
    for t in range(133):
        for k in range(128):
            dAh[t, k, k] = wA16[k, t]
            dAl[t, k, k] = np.float16(wA16[k, t].astype(np.float64) / 64.0)
            dAr[t, k, k] = np.float16(wA[k, t] - wA16[k, t].astype(np.float64))
    w["dgAh"] = dAh
    w["dgAl"] = dAl
    w["dgAr"] = dAr
    w["poswr"] = (wA - wA16.astype(np.float64)).astype(np.float32)
    for i, nm in enumerate(["W1big", "W2big", "W3big"], 1):
        w[f"cbias{i}"] = np.ascontiguousarray(w[nm][200]).reshape(200, 1)

    # ---- VQ tables (host-precomputed; pure weight transforms) ----
    iw = np.asarray(inp["inp_w"]).astype(np.float64)        # [LLM, DM]
    cb = np.asarray(inp["codebook"]).astype(np.float64)     # [KC, LLM]
    CB2 = iw.T @ cb.T                                        # [DM, KC]
    hi = CB2.astype(np.float16)
    lo = (CB2 - hi.astype(np.float64)).astype(np.float16)
    w["cb2hA"] = np.ascontiguousarray(hi[:128])
    w["cb2lA"] = np.ascontiguousarray(lo[:128])

    # norm rows: nvec2 = inp_b.c - 0.5|c|^2, 4-way fp16 split
    nvec2 = cb @ np.asarray(inp["inp_b"]).astype(np.float64) - 0.5 * (cb * cb).sum(-1)
    n1 = nvec2.astype(np.float16).astype(np.float64)
    r = nvec2 - n1
    n2 = r.astype(np.float16).astype(np.float64)
    r = r - n2
    n3 = r.astype(np.float16).astype(np.float64)
    n4 = r - n3
    # B tiles: rows 0..71 = dm 128..199, 72..95 = zeros, 96..97 = norm rows
    # (32-aligned partition base for the ones-rows memsets in pe16 tiles)
    hB = np.zeros((98, KC), np.float16)
    lB = np.zeros((98, KC), np.float16)
    hB[:72] = hi[128:200]
    lB[:72] = lo[128:200]
    hB[96], hB[97] = n1.astype(np.float16), n3.astype(np.float16)
    lB[96], lB[97] = n2.astype(np.float16), n4.astype(np.float16)
    w["cb2hB"] = hB
    w["cb2lB"] = lB

    w2 = cb @ np.asarray(inp["outp_w"]).astype(np.float64).T \
        + np.asarray(inp["outp_b"]).astype(np.float64)
    w["w2f"] = w2.astype(np.float16)                         # [KC, DM]
    return w


def _build_nc(debug=False):
    from contextlib import ExitStack
    import concourse.bass as bass
    import concourse.mybir as mybir
    import concourse.tile as tile
    from concourse import bacc

    f32 = mybir.dt.float32
    f16 = mybir.dt.float16
    u32 = mybir.dt.uint32
    Alu = mybir.AluOpType
    AF = mybir.ActivationFunctionType
    AX = mybir.AxisListType.X

    nc = bacc.Bacc("TRN2", target_bir_lowering=False, debug=False, num_devices=NCORES)

    di = {}
    di["xT"] = nc.dram_tensor("xT", [200, TOK], f32, kind="ExternalInput")
    for nm in ["W1big", "W2big", "W3big"]:
        di[nm] = nc.dram_tensor(nm, [201, 200], f32, kind="ExternalInput")
    di["Fcat"] = nc.dram_tensor("Fcat", [201, 202], f32, kind="ExternalInput")
    di["spec_wT"] = nc.dram_tensor("spec_wT", [102, 200], f32, kind="ExternalInput")
    for i in range(1, 4):
        di[f"gn{i}gamma"] = nc.dram_tensor(f"gn{i}gamma", [200, 1], f32, kind="ExternalInput")
        di[f"gn{i}beta"] = nc.dram_tensor(f"gn{i}beta", [200, 1], f32, kind="ExternalInput")
    di["gmask"] = nc.dram_tensor("gmask", [200, 5], f32, kind="ExternalInput")
    di["gmaskT"] = nc.dram_tensor("gmaskT", [5, 200], f32, kind="ExternalInput")
    di["posw"] = nc.dram_tensor("posw", [200, 133], f32, kind="ExternalInput")
    for nm in ["dgAh", "dgAl", "dgAr"]:
        di[nm] = nc.dram_tensor(nm, [133, 128, 128], f16, kind="ExternalInput")
    di["poswr"] = nc.dram_tensor("poswr", [128, 133], f32, kind="ExternalInput")
    for i in range(1, 4):
        di[f"cbias{i}"] = nc.dram_tensor(f"cbias{i}", [200, 1], f32, kind="ExternalInput")
    di["posb"] = nc.dram_tensor("posb", [200, 1], f32, kind="ExternalInput")
    di["cb2hA"] = nc.dram_tensor("cb2hA", [128, KC], f16, kind="ExternalInput")
    di["cb2lA"] = nc.dram_tensor("cb2lA", [128, KC], f16, kind="ExternalInput")
    di["cb2hB"] = nc.dram_tensor("cb2hB", [98, KC], f16, kind="ExternalInput")
    di["cb2lB"] = nc.dram_tensor("cb2lB", [98, KC], f16, kind="ExternalInput")
    di["w2f"] = nc.dram_tensor("w2f", [KC, DM], f16, kind="ExternalInput")

    out_d = nc.dram_tensor("out", [TOK, 200], f16, kind="ExternalOutput")
    idx_d = nc.dram_tensor("idx", [128, 18], u32, kind="ExternalOutput")
    dbg = {}
    if debug:
        for nm in ["d_pe", "d_g1", "d_pe1"]:
            dbg[nm] = nc.dram_tensor(nm, [200, TOK], f32, kind="ExternalOutput")

    TT = _tok_tiles()
    NS = _n_slices()
    PSH = [128, NB * PBLK]     # padded_shape for pos-conv-sized fe2 tags

    with tile.TileContext(nc) as tc:
        late = ExitStack()
        with late, (
            tc.tile_pool(name="persist", bufs=1)) as persist, (
            tc.tile_pool(name="pconst", bufs=1)) as pconst, (
            tc.tile_pool(name="mid", bufs=1)) as mid:
            gidxu = persist.tile([128, 18], u32, name="gidxu")
            onesT = pconst.tile([1, 512], f32, name="onesT")
            nc.vector.memset(onesT[:], 1.0)
            z16 = pconst.tile([1, 128], f16, name="z16")
            nc.vector.memset(z16[:], 0.0)
            ones512h = pconst.tile([1, 512], f16, name="ones512h")
            nc.vector.memset(ones512h[:], 1.0)

            # score tables (host-precomputed), loaded once
            cb2hA = mid.tile([128, KC], f16, name="cb2hA")
            cb2lA = mid.tile([128, KC], f16, name="cb2lA")
            cb2hB = mid.tile([98, KC], f16, name="cb2hB")
            cb2lB = mid.tile([98, KC], f16, name="cb2lB")
            nc.sync.dma_start(cb2hA[:], di["cb2hA"][:])
            nc.sync.dma_start(cb2lA[:], di["cb2lA"][:])
            nc.sync.dma_start(cb2hB[:], di["cb2hB"][:])
            nc.sync.dma_start(cb2lB[:], di["cb2lB"][:])

            # ---------------- Front end ----------------
            with (
                tc.tile_pool(name="fe2", bufs=1) as fe2,
                tc.tile_pool(name="fetmp", bufs=2) as fetmp,
            ):
                gmA = pconst.tile([128, 5], f32, name="gmA")
                gmB = pconst.tile([72, 5], f32, name="gmB")
                gmT = pconst.tile([5, 200], f32, name="gmT")
                nc.sync.dma_start(gmA[:], di["gmask"][0:128, :])
                nc.sync.dma_start(gmB[:], di["gmask"][128:200, :])
                nc.sync.dma_start(gmT[:], di["gmaskT"][:])

                g1A = fe2.tile([128, TOK], f32, name="g1A", tag="gA1", padded_shape=PSH)
                g1B = fe2.tile([72, TOK], f32, name="g1B", tag="gB1", padded_shape=PSH)
                g2A = fe2.tile([128, TOK], f32, name="g2A", tag="gA2")
                g2B = fe2.tile([72, TOK], f32, name="g2B", tag="gB2")
                g3A = fe2.tile([128, TOK], f32, name="g3A", tag="gA1", padded_shape=PSH)
                g3B = fe2.tile([72, TOK], f32, name="g3B", tag="gB1", padded_shape=PSH)
                pe1A = fe2.tile([128, TOK], f32, name="pe1A", tag="gA2")
                pe1B = fe2.tile([72, TOK], f32, name="pe1B", tag="gB2")

                def conv_gn_gelu(fe1, feps, stps, rhsA, rhsB, wname, gi, outA, outB,
                                 dbg_g=None):
                    """rhs [128,TOK]/[72,TOK] + onesT -> g = 2*gelu(GN(conv))."""
                    WA = fetmp.tile([128, 200], f32, name=f"WA{gi}", tag="WA")
                    WB = fetmp.tile([72, 200], f32, name=f"WB{gi}", tag="WB")
                    cbA = fetmp.tile([128, 1], f32, name=f"cbA{gi}", tag="cbA")
                    cbB = fetmp.tile([72, 1], f32, name=f"cbB{gi}", tag="cbB")
                    nc.sync.dma_start(WA[:], di[wname][0:128, :])
                    nc.sync.dma_start(WB[:], di[wname][128:200, :])
                    nc.sync.dma_start(cbA[:], di[f"cbias{gi}"][0:128, :])
                    nc.sync.dma_start(cbB[:], di[f"cbias{gi}"][128:200, :])
                    gam = fetmp.tile([128, 2], f32, name=f"gam{gi}", tag="gam")
                    bet = fetmp.tile([128, 2], f32, name=f"bet{gi}", tag="bet")
                    nc.sync.dma_start(gam[0:128, 0:1], di[f"gn{gi}gamma"][0:128, :])
                    nc.sync.dma_start(gam[0:72, 1:2], di[f"gn{gi}gamma"][128:200, :])
                    nc.sync.dma_start(bet[0:128, 0:1], di[f"gn{gi}beta"][0:128, :])
                    nc.sync.dma_start(bet[0:72, 1:2], di[f"gn{gi}beta"][128:200, :])

                    convA = fe1.tile([128, TOK], f32, name=f"convA{gi}", tag="convA")
                    convB = fe1.tile([72, TOK], f32, name=f"convB{gi}", tag="convB")
                    for (m0, msz, cdst, cbv) in [(0, 128, convA, cbA), (128, 72, convB, cbB)]:
                        for (n0, nsz) in NS:
                            cps = feps.tile([128, 512], f32, name="cps", tag="cps")
                            nc.tensor.matmul(cps[:msz, :nsz], WA[:, m0:m0 + msz],
                                             rhsA[:, n0:n0 + nsz], start=True, stop=False)
                            nc.tensor.matmul(cps[:msz, :nsz], WB[:, m0:m0 + msz],
                                             rhsB[:, n0:n0 + nsz], start=False, stop=True)
                            nc.scalar.activation(cdst[:, n0:n0 + nsz], cps[:msz, :nsz],
                                                 AF.Identity, bias=cbv[:msz, 0:1])

                    stA = fetmp.tile([128, 8], f32, name=f"stA{gi}", tag="stA")
                    stB = fetmp.tile([72, 8], f32, name=f"stB{gi}", tag="stB")
                    sqA = fe2.tile([128, TOK], f32, name=f"sqA{gi}", tag="sqA", padded_shape=PSH)
                    sqB = fe2.tile([72, TOK], f32, name=f"sqB{gi}", tag="sqB", padded_shape=PSH)
                    nc.scalar.square(sqA[:], convA[:])
                    nc.scalar.square(sqB[:], convB[:])
                    for b in range(NB):
                        sl = slice(b * T1, (b + 1) * T1)
                        nc.vector.reduce_sum(stA[:, 2 * b:2 * b + 1], convA[:, sl], axis=AX)
                        nc.vector.reduce_sum(stA[:, 2 * b + 1:2 * b + 2], sqA[:, sl], axis=AX)
                        nc.vector.reduce_sum(stB[:, 2 * b:2 * b + 1], convB[:, sl], axis=AX)
                        nc.vector.reduce_sum(stB[:, 2 * b + 1:2 * b + 2], sqB[:, sl], axis=AX)
                    sps = stps.tile([5, 8], f32, name="sps", tag="stp")
                    nc.tensor.matmul(sps[:], gmA[:], stA[:], start=True, stop=False)
                    nc.tensor.matmul(sps[:], gmB[:], stB[:], start=False, stop=True)

                    st = fetmp.tile([5, 16], f32, name=f"st{gi}", tag="st")
                    st2 = fetmp.tile([5, 8], f32, name=f"st2{gi}", tag="st2")
                    NINV = 1.0 / (40 * T1)
                    nc.vector.tensor_scalar(st[:, 0:8], sps[:], NINV, None, op0=Alu.mult)
                    for b in range(NB):
                        nc.vector.tensor_copy(st2[:, b:b + 1], st[:, 2 * b:2 * b + 1])
                        nc.vector.tensor_mul(st[:, 8 + b:9 + b], st[:, 2 * b:2 * b + 1],
                                             st[:, 2 * b:2 * b + 1])
                        nc.vector.tensor_sub(st2[:, 4 + b:5 + b], st[:, 2 * b + 1:2 * b + 2],
                                             st[:, 8 + b:9 + b])
                    nc.vector.tensor_scalar(st2[:, 4:8], st2[:, 4:8], EPS, None, op0=Alu.add)
                    sqr = fetmp.tile([5, 4], f32, name=f"sqr{gi}", tag="sqr")
                    nc.scalar.activation(sqr[:], st2[:, 4:8], AF.Sqrt)
                    r0 = fetmp.tile([5, 4], f32, name=f"r0{gi}", tag="r0")
                    nc.vector.reciprocal(r0[:], sqr[:])
                    tn = fetmp.tile([5, 4], f32, name=f"tn{gi}", tag="tn")
                    nc.vector.tensor_mul(tn[:], r0[:], r0[:])
                    nc.vector.tensor_mul(tn[:], tn[:], st2[:, 4:8])
                    nc.vector.tensor_scalar(tn[:], tn[:], -0.5, 1.5, op0=Alu.mult, op1=Alu.add)
                    nc.vector.tensor_mul(st2[:, 4:8], r0[:], tn[:])

                    bpsA = stps.tile([128, 8], f32, name="bpsA", tag="stp")
                    bpsB = stps.tile([72, 8], f32, name="bpsB", tag="stp")
                    nc.tensor.matmul(bpsA[:], gmT[:, 0:128], st2[:], start=True, stop=True)
                    nc.tensor.matmul(bpsB[:], gmT[:, 128:200], st2[:], start=True, stop=True)
                    rgA = fetmp.tile([128, 8], f32, name=f"rgA{gi}", tag="rgA")
                    rgB = fetmp.tile([72, 8], f32, name=f"rgB{gi}", tag="rgB")
                    for (bps, rg, gcol, prt) in [(bpsA, rgA, 0, 128), (bpsB, rgB, 1, 72)]:
                        nc.vector.tensor_scalar(rg[:prt, 0:4], bps[:prt, 4:8],
                                                gam[:prt, gcol:gcol + 1], None, op0=Alu.mult)
                        nc.vector.tensor_mul(rg[:prt, 4:8], bps[:prt, 0:4], rg[:prt, 0:4])
                        nc.vector.tensor_scalar(rg[:prt, 4:8], rg[:prt, 4:8],
                                                bet[:prt, gcol:gcol + 1], None, op0=Alu.subtract)
                    zA = fe2.tile([128, TOK], f32, name=f"zA{gi}", tag="zA", padded_shape=PSH)
                    zB = fe2.tile([72, TOK], f32, name=f"zB{gi}", tag="zB", padded_shape=PSH)
                    for b in range(NB):
                        sl = slice(b * T1, (b + 1) * T1)
                        nc.vector.tensor_scalar(zA[:, sl], convA[:, sl], rgA[:, b:b + 1],
                                                rgA[:, 4 + b:5 + b], op0=Alu.mult, op1=Alu.subtract)
                        nc.vector.tensor_scalar(zB[:, sl], convB[:, sl], rgB[:, b:b + 1],
                                                rgB[:, 4 + b:5 + b], op0=Alu.mult, op1=Alu.subtract)
                    eA = fe1.tile([128, TOK], f32, name=f"eA{gi}", tag="convA")
                    eB = fe1.tile([72, TOK], f32, name=f"eB{gi}", tag="convB")
                    nc.scalar.activation(eA[:], zA[:], AF.Erf, scale=SQ2I)
                    nc.scalar.activation(eB[:], zB[:], AF.Erf, scale=SQ2I)
                    nc.vector.scalar_tensor_tensor(outA[0:128, :], eA[:], 1.0, zA[:],
                                                   op0=Alu.add, op1=Alu.mult)
                    nc.vector.scalar_tensor_tensor(outB[0:72, :], eB[:], 1.0, zB[:],
                                                   op0=Alu.add, op1=Alu.mult)
                    if dbg_g is not None:
                        nc.sync.dma_start(dbg_g[0:128, :], outA[0:128, :])
                        nc.sync.dma_start(dbg_g[128:200, :], outB[0:72, :])

                with (
                    tc.tile_pool(name="fe1", bufs=1) as fe1,
                    tc.tile_pool(name="feps", bufs=3, space="PSUM") as feps,
                    tc.tile_pool(name="stps", bufs=1, space="PSUM") as stps,
                ):
                    # --- x arrives pre-transposed: [200, TOK]
                    xA = fe1.tile([128, TOK], f32, name="xA", tag="xA")
                    xB = fe1.tile([72, TOK], f32, name="xB", tag="xB")
                    nc.sync.dma_start(xA[:], di["xT"][0:128, :])
                    nc.sync.dma_start(xB[:], di["xT"][128:200, :])

                    conv_gn_gelu(fe1, feps, stps, xA, xB, "W1big", 1, g1A, g1B,
                                 dbg.get("d_g1"))
                    conv_gn_gelu(fe1, feps, stps, g1A, g1B, "W2big", 2, g2A, g2B)
                    conv_gn_gelu(fe1, feps, stps, g2A, g2B, "W3big", 3, g3A, g3B)

                    # --- FFT + spec proj; pe1 = 0.5*g3 + specproj
                    FA = fetmp.tile([128, 202], f32, name="FA", tag="WA")
                    FB = fetmp.tile([72, 202], f32, name="FB", tag="WB")
                    nc.sync.dma_start(FA[:], di["Fcat"][0:128, :])
                    nc.sync.dma_start(FB[:], di["Fcat"][128:200, :])
                    reT = fe2.tile([101, TOK], f32, name="reT", tag="sqA", padded_shape=PSH)
                    imT = fe2.tile([101, TOK], f32, name="imT", tag="sqB", padded_shape=PSH)
                    for (m0, dst) in [(0, reT), (101, imT)]:
                        for (n0, nsz) in NS:
                            cps = feps.tile([128, 512], f32, name="cpsf", tag="cps")
                            nc.tensor.matmul(cps[:101, :nsz], FA[:, m0:m0 + 101],
                                             xA[:, n0:n0 + nsz], start=True, stop=False)
                            nc.tensor.matmul(cps[:101, :nsz], FB[:, m0:m0 + 101],
                                             xB[:, n0:n0 + nsz], start=False, stop=True)
                            nc.scalar.activation(dst[:, n0:n0 + nsz], cps[:101, :nsz], AF.Copy)
                    specA = fe1.tile([101, TOK], f32, name="specA", tag="convA")
                    nc.vector.tensor_mul(reT[:], reT[:], reT[:])
                    nc.vector.tensor_mul(imT[:], imT[:], imT[:])
                    nc.vector.tensor_add(reT[:], reT[:], imT[:])
                    epsb = fetmp.tile([101, 1], f32, name="epsb", tag="gam")
                    nc.vector.memset(epsb[:], 1e-30)
                    nc.scalar.activation(specA[0:101, :], reT[:], AF.Sqrt, bias=epsb[:])
                    swT = fetmp.tile([101, 200], f32, name="swT", tag="WB")
                    swTb = fetmp.tile([1, 200], f32, name="swTb", tag="Wb")
                    nc.sync.dma_start(swT[:], di["spec_wT"][0:101, :])
                    nc.sync.dma_start(swTb[:], di["spec_wT"][101:102, :])
                    for (m0, msz, gsrc, pdst) in [(0, 128, g3A, pe1A), (128, 72, g3B, pe1B)]:
                        for (n0, nsz) in NS:
                            cps = feps.tile([128, 512], f32, name="cpss", tag="cps")
                            nc.tensor.matmul(cps[:msz, :nsz], swT[:, m0:m0 + msz],
                                             specA[:, n0:n0 + nsz], start=True, stop=False)
                            nc.tensor.matmul(cps[:msz, :nsz], swTb[:, m0:m0 + msz],
                                             onesT[:, 0:nsz], start=False, stop=True)
                            nc.vector.scalar_tensor_tensor(
                                pdst[:, n0:n0 + nsz], gsrc[:msz, n0:n0 + nsz], 0.5,
                                cps[:msz, :nsz], op0=Alu.mult, op1=Alu.add)
                    if debug:
                        nc.sync.dma_start(dbg["d_pe1"][0:128, :], pe1A[:])
                        nc.sync.dma_start(dbg["d_pe1"][128:200, :], pe1B[:])

                # pe'' tiles: pool opened after fe1 freed its space,
                # closed (via `late`) after the scores phase.
                pe16 = late.enter_context(tc.tile_pool(name="pe16", bufs=1, side="right"))
                pehA = pe16.tile([128, TOK], f16, name="pehA")
                pelA = pe16.tile([128, TOK], f16, name="pelA")
                pehB = pe16.tile([98, TOK], f16, name="pehB")
                pelB = pe16.tile([98, TOK], f16, name="pelB")
                peA = pe16.tile([128, TOK], f32, name="peA")
                peB = pe16.tile([72, TOK], f32, name="peB")
                nc.vector.memset(pehB[64:98, :], 0.0)
                nc.vector.memset(pehB[96:98, :], 1.0)
                nc.vector.memset(pelB[64:98, :], 0.0)

                # --- pos conv: A group (ch 0..127) on the PE as per-tap
                # diagonal f16 matmuls (K=M=128 -> fast-weight-load) with fp32
                # PSUM accumulation, three passes: hi (w16 x pad_hi), lo
                # (w16/64 x 64*pad_lo), wres ((w-w16) denormal-f16 x pad_hi)
                # -> ~1e-6 exact. B group (ch 128..199) as an exact fp32 DVE
                # MAC chain with true weights. Both run concurrently.
                pwB = fetmp.tile([72, 133], f32, name="pwB", tag="pwB")
                nc.sync.dma_start(pwB[:], di["posw"][128:200, :])
                pwAr = fetmp.tile([128, 133], f32, name="pwAr", tag="WA")
                nc.sync.dma_start(pwAr[:], di["poswr"][0:128, :])
                pbA = fetmp.tile([128, 1], f32, name="pbA", tag="cbA")
                pbB = fetmp.tile([72, 1], f32, name="pbB", tag="cbB")
                nc.sync.dma_start(pbA[:], di["posb"][0:128, :])
                nc.sync.dma_start(pbB[:], di["posb"][128:200, :])

                padhA = fe2.tile([128, NB, PBLK], f16, name="padhA", tag="zA")
                padlA = fe2.tile([128, NB, PBLK], f16, name="padlA", tag="sqA")
                padB = fe2.tile([72, NB, PBLK], f32, name="padB", tag="zB")
                accB = fe2.tile([72, NB, PBLK], f32, name="accB", tag="gB1")
                accWr = fe2.tile([128, NB, PBLK], f32, name="accWr", tag="gA1")
                h16A = fe2.tile([128, TOK], f16, name="h16A", tag="h16A")
                lo16A = fe2.tile([128, TOK], f16, name="lo16A", tag="lo16A")
                lo32 = fe2.tile([128, TOK], f32, name="lo32", tag="gA1")
                nc.vector.memset(padhA[:], 0.0)
                nc.vector.memset(padlA[:], 0.0)
                nc.vector.memset(padB[:], 0.0)
                nc.scalar.activation(h16A[:], pe1A[:], AF.Copy)
                nc.vector.tensor_sub(lo32[:], pe1A[:], h16A[:])
                nc.vector.tensor_scalar(lo16A[:], lo32[:], 64.0, None, op0=Alu.mult)
                for b in range(NB):
                    for (ptile, flat, prt) in [(padhA, h16A, 128), (padlA, lo16A, 128),
                                               (padB, pe1B, 72)]:
                        dst = ptile[:, b, 4:688].rearrange("p (h w) -> p h w", w=36)
                        nc.scalar.activation(
                            dst[:, :, 3:33],
                            flat[:prt, b * T1:(b + 1) * T1]
                            .rearrange("p (h w) -> p h w", w=30),
                            AF.Copy)

                # B: DVE fp32 MAC chain (first tap overwrites, no acc memset)
                for i, (dy, dx, sb, db, L) in enumerate(_pos_taps()):
                    tap = dy * 7 + dx
                    srcw = padB[:, :, sb:sb + L]
                    dst = accB[:, :, db:db + L]
                    if i == 0:
                        nc.vector.tensor_scalar(dst, srcw, pwB[:, tap:tap + 1], None,
                                                op0=Alu.mult)
                    else:
                        nc.vector.scalar_tensor_tensor(dst, srcw, pwB[:, tap:tap + 1],
                                                       dst, op0=Alu.mult, op1=Alu.add)
                # A wres tail (last taps) on DVE: fills DVE slack, shrinks the
                # PE wres pass. First subset tap is dy=9 (full acc coverage).
                _all = _pos_taps()
                wres_dve = [_all[6]] + _all[97:]
                wres_pe = [t for t in _all if t not in wres_dve]
                for i, (dy, dx, sb, db, L) in enumerate(wres_dve):
                    tap = dy * 7 + dx
                    srcw = padhA[:, :, sb:sb + L]
                    dst = accWr[:, :, db:db + L]
                    if i == 0:
                        nc.vector.tensor_scalar(dst, srcw, pwAr[:, tap:tap + 1], None,
                                                op0=Alu.mult)
                    else:
                        nc.vector.scalar_tensor_tensor(dst, srcw, pwAr[:, tap:tap + 1],
                                                       dst, op0=Alu.mult, op1=Alu.add)

                # A: PE passes
                def bank_chunks(d0, L):
                    out, c = [], d0
                    while c < d0 + L:
                        e = min((c // 512 + 1) * 512, d0 + L)
                        out.append((c, e - c))
                        c = e
                    return out

                with (
                    tc.tile_pool(name="posps", bufs=1, space="PSUM") as posps,
                    tc.tile_pool(name="dgp", bufs=4) as dgp,
                ):
                    pps = posps.tile([128, 2736], f32, name="pps")
                    for c in range(0, 2736, 512):
                        n = min(512, 2736 - c)
                        nc.tensor.matmul(pps[:128, c:c + n], z16[:, 0:128],
                                         ones512h[:, 0:n], start=True, stop=False)
                    for (dgd, rhs, tlist) in [(di["dgAh"], padhA, _all),
                                              (di["dgAl"], padlA, _all),
                                              (di["dgAr"], padhA, wres_pe)]:
                        for (dy, dx, sbg, dbg_, L) in tlist:
                            d = dy - 9
                            ho, hn, hs = max(0, -d), 19 - abs(d), max(0, d)
                            shift = (hs - ho) * 36 + dx - 3
                            dg = dgp.tile([128, 128], f16, name="dg", tag="dg")
                            nc.sync.dma_start(dg[:], dgd[dy * 7 + dx, :, :])
                            for b in range(NB):
                                d0 = b * 684 + ho * 36
                                for (c, n) in bank_chunks(d0, hn * 36):
                                    sb0 = 4 + (c - b * 684) + shift
                                    nc.tensor.matmul(
                                        pps[:128, c:c + n], dg[:, :],
                                        rhs[:, b, sb0:sb0 + n],
                                        start=False, stop=False)
                    for c in range(0, 2736, 512):
                        n = min(512, 2736 - c)
                        nc.tensor.matmul(pps[:128, c:c + n], z16[:, 0:128],
                                         ones512h[:, 0:n], start=False, stop=True)
                    # assemblies: pe'' = (acc + posb) + pe1; then f16 hi/lo split
                    for (prt, accv_fn, wr, pb, src_, pe, peh, pel) in [
                            (72, lambda b: accB[:, b, 4:688]
                             .rearrange("p (h w) -> p h w", w=36), None, pbB, pe1B,
                             peB, pehB, pelB),
                            (128, lambda b: pps[:128, b * 684:(b + 1) * 684]
                             .rearrange("p (h w) -> p h w", w=36), accWr, pbA, pe1A,
                             peA, pehA, pelA)]:
                        for b in range(NB):
                            ov = pe[:prt, b * T1:(b + 1) * T1] \
                                .rearrange("p (h w) -> p h w", w=30)
                            nc.vector.scalar_tensor_tensor(
                                ov, accv_fn(b)[:, :, 3:33], pb[:, 0:1],
                                src_[:prt, b * T1:(b + 1) * T1]
                                .rearrange("p (h w) -> p h w", w=30),
                                op0=Alu.add, op1=Alu.add)
                            if wr is not None:
                                wv = wr[:, b, 4:688].rearrange(
                                    "p (h w) -> p h w", w=36)
                                nc.vector.tensor_add(ov, ov, wv[:, :, 3:33])
                        nc.scalar.activation(peh[:prt, :], pe[:prt, :], AF.Copy)
                        nc.vector.tensor_sub(pel[:prt, :], pe[:prt, :], peh[:prt, :])
                if debug:
                    nc.sync.dma_start(dbg["d_pe"][0:128, :], peA[:])
                    nc.sync.dma_start(dbg["d_pe"][128:200, :], peB[:])

            # ------- scores (tok-tile outer) + argmax + indirect gather
            with (
                tc.tile_pool(name="sce", bufs=2) as sce,
                tc.tile_pool(name="gat", bufs=3) as gat,
                tc.tile_pool(name="scps", bufs=4, space="PSUM") as scps,
            ):
                for ti, (t0, tsz) in enumerate(TT):
                    tsl = slice(t0, t0 + tsz)
                    sc = sce.tile([128, KC], f32, name="sc", tag="sc")
                    for kc in range(8):
                        csl = slice(kc * 512, (kc + 1) * 512)
                        sps_ = scps.tile([128, 512], f32, name="sps_", tag="sps")
                        seq = [
                            (pehA, cb2hA), (pehB, cb2hB),   # term1 (+norm hi)
                            (pelA, cb2hA), (pelB, cb2hB),   # term2
                            (pehA, cb2lA), (pehB, cb2lB),   # term3 (+norm lo)
                        ]
                        for i, (lh, rh) in enumerate(seq):
                            nc.tensor.matmul(sps_[:tsz, :], lh[:, tsl], rh[:, csl],
                                             start=(i == 0), stop=(i == len(seq) - 1))
                        nc.scalar.activation(sc[:tsz, csl], sps_[:tsz, :], AF.Copy)
                    mv8 = gat.tile([128, 8], f32, name="mv8", tag="mv8")
                    mi8 = gat.tile([128, 8], u32, name="mi8", tag="mi8")
                    nc.vector.max_with_indices(mv8[:tsz, :], mi8[:tsz, :], sc[:tsz, :])
                    nc.vector.tensor_copy(gidxu[:tsz, ti:ti + 1], mi8[:tsz, 0:1])
                    grow = gat.tile([128, 200], f16, name="grow", tag="grow")
                    nc.gpsimd.indirect_dma_start(
                        out=grow[:tsz, :], out_offset=None,
                        in_=di["w2f"][:],
                        in_offset=bass.IndirectOffsetOnAxis(ap=mi8[:tsz, 0:1], axis=0))
                    nc.sync.dma_start(out_d[t0:t0 + tsz, :], grow[:tsz, :])
                nc.sync.dma_start(idx_d[:], gidxu[:])

    nc.compile()
    return nc


def _prep_inputs(inp):
    w = build_host_weights(inp)
    x = np.asarray(inp["x"], np.float32).reshape(B * T1, 200)
    in_maps = []
    for c in range(NCORES):
        m = {"xT": np.ascontiguousarray(x[c * TOK:(c + 1) * TOK].T)}
        for k in ["W1big", "W2big", "W3big", "Fcat", "spec_wT", "gmask", "gmaskT",
                  "posw", "posb", "poswr", "dgAh", "dgAl", "dgAr", "cb2hA", "cb2lA", "cb2hB", "cb2lB", "w2f",
                  "cbias1", "cbias2", "cbias3"]:
            m[k] = np.ascontiguousarray(w[k])
        for i in range(1, 4):
            m[f"gn{i}gamma"] = np.ascontiguousarray(w[f"gn{i}gamma"])
            m[f"gn{i}beta"] = np.ascontiguousarray(w[f"gn{i}beta"])
        in_maps.append(m)
    return in_maps


def run(inp, debug=False, trace=False, **kw):
    global _COMPILED
    from concourse.bass_utils import run_bass_kernel_spmd
    if _COMPILED is None or _COMPILED[1] != debug:
        _COMPILED = (_build_nc(debug=debug), debug)
    nc = _COMPILED[0]
    in_maps = _prep_inputs(inp)
    res = run_bass_kernel_spmd(nc, in_maps, core_ids=list(range(NCORES)), trace=trace, **kw)
    return res


def kernel(**inputs):
    res = run(inputs)
    out = np.concatenate([np.asarray(r["out"], np.float32) for r in res.results], 0)
    return out.reshape(B, CH, NP_, DM)
